# revision 35
# baseline (speedup 1.0000x reference)
"""Trainium2 Bass kernel for nn_CapsuleLayer (grouped 5x5 capsule conv + 3-iter
dynamic routing with local softmax), data-parallel over batch N=8 across 8 cores.

Layout: spatial positions on SBUF partitions, channels on free dims.
  hw = hb*128 + p  (raster order), hb in [0,18), p in [0,128)
  uhat: [p=128, (hb=18, ci=8, do=16, co=16)] bf16.  co innermost keeps packed
  bf16 tensor_tensor ops in the DVE 2x perf mode; broadcasts of r (over do)
  and v (over ci) are middle-dim stride-0, which preserves the fast mode.

Conv: host-side im2col (pure layout transform) stages tap-expanded lhsT pats
in DRAM; per ci one [128,HW] + one [72,HW] load, then per hb two matmuls
(K=128 taps*di, K=72) accumulate in PSUM; evacuation rotates Act/DVE/Pool.

Routing: all channel contractions are free-dim tensor-op trees in bf16, with
each big op range-split between DVE and Pool (gpsimd) so both engines run in
parallel.  The 5x5 spatial pools run in an h-on-partitions layout: one DMA
reorder down+up per side, the separable 5-tap window max/sum done with
partition-shifted (h) and free-shifted (w) tensor_tensor trees.
"""

import numpy as np
import ml_dtypes
from contextlib import ExitStack

import concourse.bass as bass
import concourse.tile as tile
from concourse import bacc, mybir
from concourse.bass_utils import run_bass_kernel_spmd

F32 = mybir.dt.float32
BF16 = mybir.dt.bfloat16
AF = mybir.ActivationFunctionType
ALU = mybir.AluOpType

CI, DI, CO, DO = 8, 8, 16, 16
H = W = 48
HW = H * W
HB = 18
ROUTING = 3
NEG = -3.0e38

# hb chunks for the big ops; within each chunk the last POOL_P hb go to the
# Pool engine (gpsimd), the rest to DVE.
CHUNKS = [(0, 6), (6, 12), (12, 18)]
CH_POOL = {0: 1, 6: 1, 12: 1}
POOL_P = 1


def _emit(nc):
    p1_d = nc.dram_tensor("p1", [CI, 128, HW], BF16, kind="ExternalInput").ap()
    p2_d = nc.dram_tensor("p2", [CI, 72, HW], BF16, kind="ExternalInput").ap()
    w1_d = nc.dram_tensor("w1", [128, CI, 256], BF16, kind="ExternalInput").ap()
    w2_d = nc.dram_tensor("w2", [72, CI, 256], BF16, kind="ExternalInput").ap()
    r0_d = nc.dram_tensor("r0c", [128, HB], F32, kind="ExternalInput").ap()
    v_d = nc.dram_tensor("v", [128, HB, DO, CO], BF16, kind="ExternalOutput").ap()

    with tile.TileContext(nc) as tc, ExitStack() as ctx:
        const = ctx.enter_context(tc.tile_pool(name="const", bufs=1))
        patp = ctx.enter_context(tc.tile_pool(name="patp", bufs=1))
        psum = ctx.enter_context(tc.tile_pool(name="psum", bufs=8, space="PSUM"))
        big = ctx.enter_context(tc.tile_pool(name="big", bufs=1))
        scr = ctx.enter_context(tc.tile_pool(name="scr", bufs=1))
        sm = ctx.enter_context(tc.tile_pool(name="sm", bufs=1))
        poolt = ctx.enter_context(tc.tile_pool(name="poolt", bufs=1))
        dpool = ctx.enter_context(tc.tile_pool(name="dpool", bufs=2, space="DRAM"))

        # ---- persistent tiles ----
        uhat = big.tile([128, HB, CI, DO, CO], BF16, name="uhat")
        b_t = big.tile([128, HB, CI, CO], F32, name="b_t")
        p_t = big.tile([128, HB, DO, CO], BF16, name="p_t")
        v_bf = big.tile([128, HB, DO, CO], BF16, name="v_bf")
        c_t = big.tile([128, HB, CI, CO], BF16, name="c_t")
        w1_t = const.tile([128, CI, 256], BF16, name="w1_t")
        w2_t = const.tile([72, CI, 256], BF16, name="w2_t")
        r0_t = const.tile([128, HB], F32, name="r0_t")
        eps_t = const.tile([128, 1], F32, name="eps_t")
        nc.sync.dma_start(w1_t[:], w1_d[:])
        nc.sync.dma_start(w2_t[:], w2_d[:])
        nc.vector.memset(eps_t[:], 1e-9)

        # pool scratch.  W direction runs h-on-partitions with free-dim
        # shifts; H direction via 5 row-shifted DRAM re-reads (engines cannot
        # shift across partitions).  wp w-pad columns re-set per call.
        wp = poolt.tile([48, 52, CI], F32, name="wp")
        wt1 = poolt.tile([48, 51, CI], F32, name="wt1", tag="w1")
        wt2 = poolt.tile([48, 49, CI], F32, name="wt2", tag="w2")
        wt3 = poolt.tile([48, 48, CI], F32, name="wt3", tag="w3")
        hsh = poolt.tile([128, 5, HB, CI], F32, name="hsh")
        q1 = poolt.tile([128, HB, CI], F32, name="q1", tag="q1")
        q2 = poolt.tile([128, HB, CI], F32, name="q2", tag="q2")
        # DRAM row-padded buffers (2+48+2 rows) with guard rows written once
        gpad = poolt.tile([96, CI], F32, name="gpad")
        mdBM = dpool.tile([52 * W, CI], F32, name="mdBM", tag="mdBM", bufs=1)
        mdBS = dpool.tile([52 * W, CI], F32, name="mdBS", tag="mdBS", bufs=1)
        nc.vector.memset(gpad[:], NEG)
        nc.sync.dma_start(mdBM[0 : 2 * W], gpad[:])
        nc.sync.dma_start(mdBM[50 * W : 52 * W], gpad[:])
        nc.vector.memset(gpad[:], 0.0)
        nc.sync.dma_start(mdBS[0 : 2 * W], gpad[:])
        nc.sync.dma_start(mdBS[50 * W : 52 * W], gpad[:])

        # small persistent maps
        m0_t = sm.tile([128, HB, CI], F32, name="m0_t")
        bmax_t = sm.tile([128, HB, CI], F32, name="bmax_t")
        s_t = sm.tile([128, HB, CI], F32, name="s_t")
        sumc_t = sm.tile([128, HB, CI], F32, name="sumc_t")
        dum = sm.tile([128, 1], F32, name="dum")
        rcp_t = sm.tile([128, HB, CI], F32, name="rcp_t")
        rcpb_t = sm.tile([128, HB, CI], BF16, name="rcpb_t")
        nsq_t = sm.tile([128, HB, CO], F32, name="nsq_t")
        rs_t = sm.tile([128, HB, CO], F32, name="rs_t")
        g2b_t = sm.tile([128, HB, CO], BF16, name="g2b_t")

        S_t = v_bf  # v_bf is free until the final iteration

        # =========== Stage 1: conv -> uhat ===========
        EVAC = [nc.scalar, nc.vector]
        for ci in range(CI):
            pat1 = [None, None]
            pat2 = [None, None]
            for hlf in range(2):
                pat1[hlf] = patp.tile([128, HW // 2], BF16,
                                      name=f"pat1{hlf}", tag="pat1", bufs=2)
                pat2[hlf] = patp.tile([72, HW // 2], BF16,
                                      name=f"pat2{hlf}", tag="pat2", bufs=2)
                nc.sync.dma_start(pat1[hlf][:],
                                  p1_d[ci, :, hlf * 1152 : (hlf + 1) * 1152])
                nc.sync.dma_start(pat2[hlf][:],
                                  p2_d[ci, :, hlf * 1152 : (hlf + 1) * 1152])
            for hp in range(HB // 2):
                ps = psum.tile([128, 2, 256], F32, name="ps", tag="ps", bufs=4)
                for k in range(2):
                    hb = hp * 2 + k
                    hlf, cb = divmod(hb * 128, 1152)
                    lhs1 = pat1[hlf][:, cb : cb + 128]
                    lhs2 = pat2[hlf][:, cb : cb + 128]
                    nc.tensor.matmul(
                        ps[:, k], lhs1, w1_t[:, ci, :], start=True, stop=False
                    )
                    nc.tensor.matmul(
                        ps[:, k], lhs2, w2_t[:, ci, :], start=False, stop=True
                    )
                eng = EVAC[hp % 2]
                dst = uhat[:, 2 * hp : 2 * hp + 2, ci]
                src = ps[:].rearrange("p k (d c) -> p k d c", d=DO)
                if eng is nc.scalar:
                    nc.scalar.copy(dst, src)
                else:
                    eng.tensor_copy(dst, src)
            # running S = sum_ci uhat (hidden in conv slack)
            if ci == 0:
                nc.vector.tensor_copy(S_t[:, 0:9], uhat[:, 0:9, 0])
                nc.gpsimd.tensor_copy(S_t[:, 9:18], uhat[:, 9:18, 0])
            else:
                nc.vector.tensor_tensor(
                    S_t[:, 0:12], S_t[:, 0:12], uhat[:, 0:12, ci], op=ALU.add
                )
                nc.gpsimd.tensor_tensor(
                    S_t[:, 12:18], S_t[:, 12:18], uhat[:, 12:18, ci],
                    op=ALU.add,
                )

        # =========== helpers ===========
        def parts(h0, h1, pool_hb=None):
            if pool_hb is None:
                pool_hb = CH_POOL.get(h0, POOL_P)
            d = h1 - pool_hb
            out = []
            if pool_hb:
                out.append((nc.gpsimd, d, h1, "p"))
            if d > h0:
                out.append((nc.vector, h0, d, "d"))
            return out

        def tt_split(h0, h1, dst_f, a_f, b_f, op, pool_hb=None):
            """dst = a op b over hb range [h0,h1): DVE takes [h0,h1-pool_hb),
            Pool the rest.  *_f(lo,hi) -> AP view for that hb range."""
            if pool_hb is None:
                pool_hb = CH_POOL.get(h0, POOL_P)
            d = h1 - pool_hb
            if d > h0:
                nc.vector.tensor_tensor(dst_f(h0, d), a_f(h0, d), b_f(h0, d), op=op)
            if pool_hb:
                nc.gpsimd.tensor_tensor(dst_f(d, h1), a_f(d, h1), b_f(d, h1), op=op)

        def pools(src, is_max, out):
            """src [128,(hb,ci)] f32 -> 5x5 'same' window max/sum -> out."""
            op = ALU.max if is_max else ALU.add
            pad = NEG if is_max else 0.0
            mdB = mdBM if is_max else mdBS
            nc.vector.memset(wp[:, 0:2], pad)
            nc.vector.memset(wp[:, 50:52], pad)
            md = dpool.tile([HW, CI], F32, name="pmd", tag="pmd")
            nc.sync.dma_start(
                md[:].rearrange("(hb p) ci -> p hb ci", hb=HB), src[:]
            )
            nc.sync.dma_start(
                wp[:, 2:50, :], md[:].rearrange("(h w) ci -> h w ci", h=H)
            )
            # w-direction 5-tap tree (free-dim shifts)
            nc.vector.tensor_tensor(wt1[:], wp[:, 0:51], wp[:, 1:52], op=op)
            nc.vector.tensor_tensor(wt2[:], wt1[:, 0:49], wt1[:, 2:51], op=op)
            nc.vector.tensor_tensor(wt3[:], wt2[:, 0:48], wp[:, 4:52], op=op)
            # h-direction: write rows into the padded DRAM buffer, read back 5
            # row-shifted copies, reduce.
            nc.sync.dma_start(
                mdB[2 * W : 50 * W].rearrange("(h w) ci -> h w ci", h=H), wt3[:]
            )
            for k in range(5):
                o = k * W
                nc.sync.dma_start(
                    hsh[:, k],
                    mdB[o : o + HW].rearrange("(hb p) ci -> p hb ci", hb=HB),
                )
            nc.vector.tensor_tensor(q1[:], hsh[:, 0], hsh[:, 1], op=op)
            nc.vector.tensor_tensor(q2[:], hsh[:, 2], hsh[:, 3], op=op)
            nc.vector.tensor_tensor(q1[:], q1[:], q2[:], op=op)
            nc.vector.tensor_tensor(out[:], q1[:], hsh[:, 4], op=op)

        # =========== Stage 2: routing ===========
        for it in range(ROUTING):
            last = it == ROUTING - 1
            if it == 0:
                for hb in range(HB):
                    nc.vector.tensor_scalar(
                        p_t[:, hb], S_t[:, hb], r0_t[:, hb : hb + 1],
                        None, op0=ALU.mult,
                    )
            else:
                pe_max_pools()
                # cs = (bmaxC - CC) - b = -(b - bmax); exp applies scale=-1
                cs = scr.tile([128, HB, CI, CO], F32, name="cs", tag="D")
                nc.vector.tensor_scalar(
                    tA[:, 0:18], m0C_t[:, 0:18], -CC, None, op0=ALU.add)
                bm_b = tA[:, 0:18].unsqueeze(3).broadcast_to(
                    [128, HB, CI, CO])
                tt_split(0, HB,
                         lambda a, b: cs[:, a:b],
                         lambda a, b: bm_b[:, a:b],
                         lambda a, b: b_t[:, a:b], ALU.subtract, pool_hb=4)
                nc.scalar.activation(c_t[:], cs[:], AF.Exp, scale=-1.0)
                s1 = scr.tile([128, HB, CI, 8], BF16, name="s1", tag="F")
                nc.vector.tensor_tensor(
                    s1[:], c_t[:, :, :, 0:8], c_t[:, :, :, 8:16], op=ALU.add
                )
                s2 = scr.tile([128, HB, CI, 4], BF16, name="s2", tag="G")
                nc.vector.tensor_tensor(
                    s2[:], s1[:, :, :, 0:4], s1[:, :, :, 4:8], op=ALU.add
                )
                s3 = scr.tile([128, HB, CI, 2], BF16, name="s3", tag="HH")
                nc.vector.tensor_tensor(
                    s3[:], s2[:, :, :, 0:2], s2[:, :, :, 2:4], op=ALU.add
                )
                nc.vector.tensor_tensor(
                    s_tb[:, 0:18], s3[:, :, :, 0], s3[:, :, :, 1], op=ALU.add
                )
                pe_sum_pools()
                nc.vector.reciprocal(rcp_t[:], sumc_t[:])
                nc.vector.tensor_copy(rcpb_t[:], rcp_t[:])
                rb = rcpb_t[:].unsqueeze(3).broadcast_to([128, HB, CI, CO])
                tt_split(0, HB,
                         lambda a, b: c_t[:, a:b],
                         lambda a, b: c_t[:, a:b],
                         lambda a, b: rb[:, a:b], ALU.mult, pool_hb=4)
                r_b = c_t[:].unsqueeze(3).broadcast_to([128, HB, CI, DO, CO])
                for (h0, h1) in CHUNKS:
                    for eng, lo, hi, kd in parts(h0, h1):
                        n = hi - lo
                        x = scr.tile([128, n, CI, DO, CO], BF16,
                                     name="x" + kd, tag="A" + kd)
                        eng.tensor_tensor(x[:], uhat[:, lo:hi], r_b[:, lo:hi],
                                          op=ALU.mult)
                        t1 = scr.tile([128, n, 4, DO, CO], BF16,
                                      name="t1b" + kd, tag="B" + kd)
                        eng.tensor_tensor(t1[:], x[:, :, 0:4], x[:, :, 4:8],
                                          op=ALU.add)
                        t2 = scr.tile([128, n, 2, DO, CO], BF16,
                                      name="t2b" + kd, tag="C" + kd)
                        eng.tensor_tensor(t2[:], t1[:, :, 0:2], t1[:, :, 2:4],
                                          op=ALU.add)
                        eng.tensor_tensor(p_t[:, lo:hi], t2[:, :, 0],
                                          t2[:, :, 1], op=ALU.add)

            # ---- squash: p -> v ----
            nc.scalar.activation(dum[:], eps_t[:], AF.Sqrt)
            for (h0, h1) in (CHUNKS if last else [(0, HB)]):
                n = h1 - h0
                sq = scr.tile([128, n, DO, CO], BF16, name="sq", tag="D")
                nc.scalar.activation(sq[:], p_t[:, h0:h1], AF.Square)
                n1 = scr.tile([128, n, 8, CO], BF16, name="n1", tag="F")
                tt_split(h0, h1,
                         lambda a, b: n1[:, a - h0 : b - h0],
                         lambda a, b: sq[:, a - h0 : b - h0, 0:8],
                         lambda a, b: sq[:, a - h0 : b - h0, 8:16], ALU.add,
                         pool_hb=0 if last else 3)
                n2 = scr.tile([128, n, 4, CO], BF16, name="n2", tag="G")
                nc.vector.tensor_tensor(n2[:], n1[:, :, 0:4], n1[:, :, 4:8],
                                        op=ALU.add)
                n3 = scr.tile([128, n, 2, CO], BF16, name="n3", tag="HH")
                nc.vector.tensor_tensor(n3[:], n2[:, :, 0:2], n2[:, :, 2:4],
                                        op=ALU.add)
                nc.vector.tensor_tensor(nsq_t[:, h0:h1], n3[:, :, 0],
                                        n3[:, :, 1], op=ALU.add)
            nc.scalar.activation(rs_t[:], nsq_t[:], AF.Sqrt, bias=eps_t[:])
            nc.vector.scalar_tensor_tensor(
                rs_t[:], nsq_t[:], 1.0, rs_t[:], op0=ALU.add, op1=ALU.mult
            )
            nc.vector.reciprocal(rs_t[:], rs_t[:])
            nc.vector.tensor_tensor(nsq_t[:], nsq_t[:], rs_t[:], op=ALU.mult)
            nc.vector.tensor_copy(g2b_t[:], nsq_t[:])
            g_b = g2b_t[:].unsqueeze(2).broadcast_to([128, HB, DO, CO])
            if last:
                tt_split(0, HB,
                         lambda a, b: v_bf[:, a:b],
                         lambda a, b: p_t[:, a:b],
                         lambda a, b: g_b[:, a:b], ALU.mult, pool_hb=4)

            if not last:
                nc.scalar.activation(dum[:], eps_t[:], AF.Exp)
                p_b = p_t[:].unsqueeze(2).broadcast_to([128, HB, CI, DO, CO])
                for (h0, h1) in CHUNKS:
                    for eng, lo, hi, kd in parts(h0, h1):
                        n = hi - lo
                        y = scr.tile([128, n, CI, DO, CO], BF16,
                                     name="y" + kd, tag="A" + kd)
                        eng.tensor_tensor(y[:], uhat[:, lo:hi], p_b[:, lo:hi],
                                          op=ALU.mult)
                        e1 = scr.tile([128, n, CI, 8, CO], BF16,
                                      name="e1" + kd, tag="B" + kd)
                        eng.tensor_tensor(e1[:], y[:, :, :, 0:8],
                                          y[:, :, :, 8:16], op=ALU.add)
                        e2 = scr.tile([128, n, CI, 4, CO], BF16,
                                      name="e2" + kd, tag="C" + kd)
                        eng.tensor_tensor(e2[:], e1[:, :, :, 0:4],
                                          e1[:, :, :, 4:8], op=ALU.add)
                        e3 = scr.tile([128, n, CI, 2, CO], BF16,
                                      name="e3" + kd, tag="J" + kd)
                        eng.tensor_tensor(e3[:], e2[:, :, :, 0:2],
                                          e2[:, :, :, 2:4], op=ALU.add)
                        db = scr.tile([128, n, CI, CO], BF16,
                                      name="db" + kd, tag="K" + kd)
                        eng.tensor_tensor(db[:], e3[:, :, :, 0],
                                          e3[:, :, :, 1], op=ALU.add)
                        g2_b = g2b_t[:, lo:hi].unsqueeze(2).broadcast_to(
                            [128, n, CI, CO])
                        if it == 0:
                            eng.tensor_tensor(b_t[:, lo:hi], db[:], g2_b,
                                              op=ALU.mult)
                        else:
                            eng.tensor_tensor(db[:], db[:], g2_b, op=ALU.mult)
                            eng.tensor_tensor(b_t[:, lo:hi], b_t[:, lo:hi],
                                              db[:], op=ALU.add)
                    n = h1 - h0
                    u1 = scr.tile([128, n, CI, 8], F32, name="u1", tag="Eu")
                    nc.vector.tensor_tensor(
                        u1[:], b_t[:, h0:h1, :, 0:8], b_t[:, h0:h1, :, 8:16],
                        op=ALU.max,
                    )
                    u2 = scr.tile([128, n, CI, 4], F32, name="u2", tag="Fu")
                    nc.vector.tensor_tensor(
                        u2[:], u1[:, :, :, 0:4], u1[:, :, :, 4:8], op=ALU.max
                    )
                    u3 = scr.tile([128, n, CI, 2], F32, name="u3", tag="Gu")
                    nc.vector.tensor_tensor(
                        u3[:], u2[:, :, :, 0:2], u2[:, :, :, 2:4], op=ALU.max
                    )
                    nc.vector.tensor_tensor(
                        m0C_t[:, h0:h1], u3[:, :, :, 0], u3[:, :, :, 1],
                        op=ALU.max,
                    )
                    nc.vector.tensor_scalar(
                        m0C_t[:, h0:h1], m0C_t[:, h0:h1], CC, None,
                        op0=ALU.add,
                    )
            else:
                for (h0, h1) in CHUNKS:
                    nc.sync.dma_start(v_d[:, h0:h1], v_bf[:, h0:h1])
    return nc


# ============================ host side ============================

_CACHE = {}


def _host_consts(w):
    # w: [Ci, Co*Do, Di, 5, 5] f32, channel index = co*16+do.
    # Conv lhsT rows: pat1 row = di*16 + kh*4 + kw (kh,kw in 0..4);
    # pat2 row = di*5 + kw for (kh=4, kw 0..5), then 40 + di*4 + kh for
    # (kh 0..4, kw=4).  Columns m = do*16 + co.
    w4 = w.reshape(CI, CO, DO, DI, 5, 5).transpose(3, 4, 5, 0, 2, 1)
    # w4: [di, kh, kw, ci, do, co]
    w4 = np.ascontiguousarray(w4).reshape(DI, 5, 5, CI, 256)
    w1 = np.ascontiguousarray(
        w4[:, 0:4, 0:4].reshape(128, CI, 256)
    ).astype(ml_dtypes.bfloat16)
    w2a = w4[:, 4, 0:5].reshape(40, CI, 256)
    w2b = w4[:, 0:4, 4].reshape(32, CI, 256)
    w2 = np.ascontiguousarray(np.concatenate([w2a, w2b], 0)).astype(
        ml_dtypes.bfloat16
    )

    hw_cnt = np.zeros((H, W), np.float32)
    for h in range(H):
        for wv in range(W):
            ch = min(h + 2, H - 1) - max(h - 2, 0) + 1
            cw = min(wv + 2, W - 1) - max(wv - 2, 0) + 1
            hw_cnt[h, wv] = ch * cw
    r0 = 1.0 / (CO * hw_cnt)
    r0c = np.ascontiguousarray(r0.reshape(HB, 128).T)
    return w1, w2, r0c


def _im2col(un):
    """un: [Ci, Di, H, W] bf16 -> pat1 [Ci, 128, HW], pat2 [Ci, 72, HW] bf16.
    Row layouts match _host_consts."""
    up = np.zeros((CI, DI, H + 4, W + 4), ml_dtypes.bfloat16)
    up[:, :, 2 : 2 + H, 2 : 2 + W] = un
    sw = np.lib.stride_tricks.sliding_window_view(up, (H, W), axis=(2, 3))
    # sw: [Ci, Di, 5, 5, H, W]
    p1 = sw[:, :, 0:4, 0:4].reshape(CI, DI * 16, HW)
    p2a = sw[:, :, 4, 0:5].reshape(CI, DI * 5, HW)
    p2b = sw[:, :, 0:4, 4].reshape(CI, DI * 4, HW)
    p2 = np.concatenate([p2a, p2b], 1)
    return np.ascontiguousarray(p1), np.ascontiguousarray(p2)


def _get_nc():
    if "nc" not in _CACHE:
        nc = bacc.Bacc("TRN2", target_bir_lowering=False, debug=False, num_devices=8)
        _emit(nc)
        nc.compile()
        _CACHE["nc"] = nc
    return _CACHE["nc"]


def kernel(u, w):
    u = np.asarray(u, np.float32)
    N = u.shape[0]
    assert N == 8
    nc = _get_nc()
    w1, w2, r0c = _host_consts(np.asarray(w, np.float32))
    ub = u.astype(ml_dtypes.bfloat16)
    in_maps = []
    for n in range(N):
        p1, p2 = _im2col(ub[n])
        in_maps.append({"p1": p1, "p2": p2, "w1": w1, "w2": w2, "r0c": r0c})
    res = run_bass_kernel_spmd(nc, in_maps, core_ids=list(range(N)))
    out = np.stack(
        [res.results[n]["v"].astype(np.float32) for n in range(N)]
    )  # [8, 128, HB, DO, CO]
    # hw = hb*128 + p ; out[n, co, do, h, w]
    out = out.transpose(0, 2, 1, 3, 4).reshape(N, HW, DO, CO)
    out = out.reshape(N, H, W, DO, CO).transpose(0, 4, 3, 1, 2)
    return np.ascontiguousarray(out, dtype=np.float32)


# revision 36
# speedup vs baseline: 1.0028x; 1.0028x over previous
"""Trainium2 Bass kernel for nn_CapsuleLayer (grouped 5x5 capsule conv + 3-iter
dynamic routing with local softmax), data-parallel over batch N=8 across 8 cores.

Layout: spatial positions on SBUF partitions, channels on free dims.
  hw = hb*128 + p  (raster order), hb in [0,18), p in [0,128)
  uhat: [p=128, (hb=18, ci=8, do=16, co=16)] bf16.  co innermost keeps packed
  bf16 tensor_tensor ops in the DVE 2x perf mode; broadcasts of r (over do)
  and v (over ci) are middle-dim stride-0, which preserves the fast mode.

Conv: host-side im2col (pure layout transform) stages tap-expanded lhsT pats
in DRAM; per ci one [128,HW] + one [72,HW] load, then per hb two matmuls
(K=128 taps*di, K=72) accumulate in PSUM; evacuation rotates Act/DVE/Pool.

Routing: all channel contractions are free-dim tensor-op trees in bf16, with
each big op range-split between DVE and Pool (gpsimd) so both engines run in
parallel.  The 5x5 spatial pools run in an h-on-partitions layout: one DMA
reorder down+up per side, the separable 5-tap window max/sum done with
partition-shifted (h) and free-shifted (w) tensor_tensor trees.
"""

import numpy as np
import ml_dtypes
from contextlib import ExitStack

import concourse.bass as bass
import concourse.tile as tile
from concourse import bacc, mybir
from concourse.bass_utils import run_bass_kernel_spmd

F32 = mybir.dt.float32
BF16 = mybir.dt.bfloat16
AF = mybir.ActivationFunctionType
ALU = mybir.AluOpType

CI, DI, CO, DO = 8, 8, 16, 16
H = W = 48
HW = H * W
HB = 18
ROUTING = 3
NEG = -3.0e38

# hb chunks for the big ops; within each chunk the last POOL_P hb go to the
# Pool engine (gpsimd), the rest to DVE.
CHUNKS = [(0, 6), (6, 12), (12, 18)]
CH_POOL = {0: 1, 6: 1, 12: 1}
POOL_P = 1


def _emit(nc):
    p1_d = nc.dram_tensor("p1", [CI, 128, HW], BF16, kind="ExternalInput").ap()
    p2_d = nc.dram_tensor("p2", [CI, 72, HW], BF16, kind="ExternalInput").ap()
    w1_d = nc.dram_tensor("w1", [128, CI, 256], BF16, kind="ExternalInput").ap()
    w2_d = nc.dram_tensor("w2", [72, CI, 256], BF16, kind="ExternalInput").ap()
    r0_d = nc.dram_tensor("r0c", [128, HB], F32, kind="ExternalInput").ap()
    v_d = nc.dram_tensor("v", [128, HB, DO, CO], BF16, kind="ExternalOutput").ap()

    with tile.TileContext(nc) as tc, ExitStack() as ctx:
        const = ctx.enter_context(tc.tile_pool(name="const", bufs=1))
        patp = ctx.enter_context(tc.tile_pool(name="patp", bufs=1))
        psum = ctx.enter_context(tc.tile_pool(name="psum", bufs=8, space="PSUM"))
        big = ctx.enter_context(tc.tile_pool(name="big", bufs=1))
        scr = ctx.enter_context(tc.tile_pool(name="scr", bufs=1))
        sm = ctx.enter_context(tc.tile_pool(name="sm", bufs=1))
        poolt = ctx.enter_context(tc.tile_pool(name="poolt", bufs=1))
        dpool = ctx.enter_context(tc.tile_pool(name="dpool", bufs=2, space="DRAM"))

        # ---- persistent tiles ----
        uhat = big.tile([128, HB, CI, DO, CO], BF16, name="uhat")
        b_t = big.tile([128, HB, CI, CO], F32, name="b_t")
        p_t = big.tile([128, HB, DO, CO], BF16, name="p_t")
        v_bf = big.tile([128, HB, DO, CO], BF16, name="v_bf")
        c_t = big.tile([128, HB, CI, CO], BF16, name="c_t")
        w1_t = const.tile([128, CI, 256], BF16, name="w1_t")
        w2_t = const.tile([72, CI, 256], BF16, name="w2_t")
        r0_t = const.tile([128, HB], F32, name="r0_t")
        eps_t = const.tile([128, 1], F32, name="eps_t")
        nc.sync.dma_start(w1_t[:], w1_d[:])
        nc.sync.dma_start(w2_t[:], w2_d[:])
        nc.sync.dma_start(r0_t[:], r0_d[:])
        nc.vector.memset(eps_t[:], 1e-9)

        # pool scratch.  W direction runs h-on-partitions with free-dim
        # shifts; H direction via 5 row-shifted DRAM re-reads (engines cannot
        # shift across partitions).  wp w-pad columns re-set per call.
        wp = poolt.tile([48, 52, CI], F32, name="wp")
        wt1 = poolt.tile([48, 51, CI], F32, name="wt1", tag="w1")
        wt2 = poolt.tile([48, 49, CI], F32, name="wt2", tag="w2")
        wt3 = poolt.tile([48, 48, CI], F32, name="wt3", tag="w3")
        hsh = poolt.tile([128, 5, HB, CI], F32, name="hsh")
        q1 = poolt.tile([128, HB, CI], F32, name="q1", tag="q1")
        q2 = poolt.tile([128, HB, CI], F32, name="q2", tag="q2")
        # DRAM row-padded buffers (2+48+2 rows) with guard rows written once
        gpad = poolt.tile([96, CI], F32, name="gpad")
        mdBM = dpool.tile([52 * W, CI], F32, name="mdBM", tag="mdBM", bufs=1)
        mdBS = dpool.tile([52 * W, CI], F32, name="mdBS", tag="mdBS", bufs=1)
        nc.vector.memset(gpad[:], NEG)
        nc.sync.dma_start(mdBM[0 : 2 * W], gpad[:])
        nc.sync.dma_start(mdBM[50 * W : 52 * W], gpad[:])
        nc.vector.memset(gpad[:], 0.0)
        nc.sync.dma_start(mdBS[0 : 2 * W], gpad[:])
        nc.sync.dma_start(mdBS[50 * W : 52 * W], gpad[:])

        # small persistent maps
        m0_t = sm.tile([128, HB, CI], F32, name="m0_t")
        bmax_t = sm.tile([128, HB, CI], F32, name="bmax_t")
        s_t = sm.tile([128, HB, CI], F32, name="s_t")
        sumc_t = sm.tile([128, HB, CI], F32, name="sumc_t")
        dum = sm.tile([128, 1], F32, name="dum")
        rcp_t = sm.tile([128, HB, CI], F32, name="rcp_t")
        rcpb_t = sm.tile([128, HB, CI], BF16, name="rcpb_t")
        nsq_t = sm.tile([128, HB, CO], F32, name="nsq_t")
        rs_t = sm.tile([128, HB, CO], F32, name="rs_t")
        g2b_t = sm.tile([128, HB, CO], BF16, name="g2b_t")

        S_t = v_bf  # v_bf is free until the final iteration

        # =========== Stage 1: conv -> uhat ===========
        EVAC = [nc.scalar, nc.vector]
        for ci in range(CI):
            pat1 = [None, None]
            pat2 = [None, None]
            for hlf in range(2):
                pat1[hlf] = patp.tile([128, HW // 2], BF16,
                                      name=f"pat1{hlf}", tag="pat1", bufs=2)
                pat2[hlf] = patp.tile([72, HW // 2], BF16,
                                      name=f"pat2{hlf}", tag="pat2", bufs=2)
                nc.sync.dma_start(pat1[hlf][:],
                                  p1_d[ci, :, hlf * 1152 : (hlf + 1) * 1152])
                nc.sync.dma_start(pat2[hlf][:],
                                  p2_d[ci, :, hlf * 1152 : (hlf + 1) * 1152])
            for hp in range(HB // 2):
                ps = psum.tile([128, 2, 256], F32, name="ps", tag="ps", bufs=4)
                for k in range(2):
                    hb = hp * 2 + k
                    hlf, cb = divmod(hb * 128, 1152)
                    lhs1 = pat1[hlf][:, cb : cb + 128]
                    lhs2 = pat2[hlf][:, cb : cb + 128]
                    nc.tensor.matmul(
                        ps[:, k], lhs1, w1_t[:, ci, :], start=True, stop=False
                    )
                    nc.tensor.matmul(
                        ps[:, k], lhs2, w2_t[:, ci, :], start=False, stop=True
                    )
                eng = EVAC[hp % 2]
                dst = uhat[:, 2 * hp : 2 * hp + 2, ci]
                src = ps[:].rearrange("p k (d c) -> p k d c", d=DO)
                if eng is nc.scalar:
                    nc.scalar.copy(dst, src)
                else:
                    eng.tensor_copy(dst, src)
            # running S = sum_ci uhat (hidden in conv slack)
            if ci == 0:
                nc.vector.tensor_copy(S_t[:, 0:9], uhat[:, 0:9, 0])
                nc.gpsimd.tensor_copy(S_t[:, 9:18], uhat[:, 9:18, 0])
            else:
                nc.vector.tensor_tensor(
                    S_t[:, 0:12], S_t[:, 0:12], uhat[:, 0:12, ci], op=ALU.add
                )
                nc.gpsimd.tensor_tensor(
                    S_t[:, 12:18], S_t[:, 12:18], uhat[:, 12:18, ci],
                    op=ALU.add,
                )

        # =========== helpers ===========
        def parts(h0, h1, pool_hb=None):
            if pool_hb is None:
                pool_hb = CH_POOL.get(h0, POOL_P)
            d = h1 - pool_hb
            out = []
            if pool_hb:
                out.append((nc.gpsimd, d, h1, "p"))
            if d > h0:
                out.append((nc.vector, h0, d, "d"))
            return out

        def tt_split(h0, h1, dst_f, a_f, b_f, op, pool_hb=None):
            """dst = a op b over hb range [h0,h1): DVE takes [h0,h1-pool_hb),
            Pool the rest.  *_f(lo,hi) -> AP view for that hb range."""
            if pool_hb is None:
                pool_hb = CH_POOL.get(h0, POOL_P)
            d = h1 - pool_hb
            if d > h0:
                nc.vector.tensor_tensor(dst_f(h0, d), a_f(h0, d), b_f(h0, d), op=op)
            if pool_hb:
                nc.gpsimd.tensor_tensor(dst_f(d, h1), a_f(d, h1), b_f(d, h1), op=op)

        def pools(src, is_max, out):
            """src [128,(hb,ci)] f32 -> 5x5 'same' window max/sum -> out."""
            op = ALU.max if is_max else ALU.add
            pad = NEG if is_max else 0.0
            mdB = mdBM if is_max else mdBS
            nc.vector.memset(wp[:, 0:2], pad)
            nc.vector.memset(wp[:, 50:52], pad)
            md = dpool.tile([HW, CI], F32, name="pmd", tag="pmd")
            nc.sync.dma_start(
                md[:].rearrange("(hb p) ci -> p hb ci", hb=HB), src[:]
            )
            nc.sync.dma_start(
                wp[:, 2:50, :], md[:].rearrange("(h w) ci -> h w ci", h=H)
            )
            # w-direction 5-tap tree (free-dim shifts)
            nc.vector.tensor_tensor(wt1[:], wp[:, 0:51], wp[:, 1:52], op=op)
            nc.vector.tensor_tensor(wt2[:], wt1[:, 0:49], wt1[:, 2:51], op=op)
            nc.vector.tensor_tensor(wt3[:], wt2[:, 0:48], wp[:, 4:52], op=op)
            # h-direction: write rows into the padded DRAM buffer, read back 5
            # row-shifted copies, reduce.
            nc.sync.dma_start(
                mdB[2 * W : 50 * W].rearrange("(h w) ci -> h w ci", h=H), wt3[:]
            )
            for k in range(5):
                o = k * W
                nc.sync.dma_start(
                    hsh[:, k],
                    mdB[o : o + HW].rearrange("(hb p) ci -> p hb ci", hb=HB),
                )
            nc.vector.tensor_tensor(q1[:], hsh[:, 0], hsh[:, 1], op=op)
            nc.vector.tensor_tensor(q2[:], hsh[:, 2], hsh[:, 3], op=op)
            nc.vector.tensor_tensor(q1[:], q1[:], q2[:], op=op)
            nc.vector.tensor_tensor(out[:], q1[:], hsh[:, 4], op=op)

        # =========== Stage 2: routing ===========
        for it in range(ROUTING):
            last = it == ROUTING - 1
            if it == 0:
                for hb in range(HB):
                    nc.vector.tensor_scalar(
                        p_t[:, hb], S_t[:, hb], r0_t[:, hb : hb + 1],
                        None, op0=ALU.mult,
                    )
            else:
                pe_max_pools()
                # cs = (bmaxC - CC) - b = -(b - bmax); exp applies scale=-1
                cs = scr.tile([128, HB, CI, CO], F32, name="cs", tag="D")
                nc.vector.tensor_scalar(
                    tA[:, 0:18], m0C_t[:, 0:18], -CC, None, op0=ALU.add)
                bm_b = tA[:, 0:18].unsqueeze(3).broadcast_to(
                    [128, HB, CI, CO])
                tt_split(0, HB,
                         lambda a, b: cs[:, a:b],
                         lambda a, b: bm_b[:, a:b],
                         lambda a, b: b_t[:, a:b], ALU.subtract, pool_hb=4)
                nc.scalar.activation(c_t[:], cs[:], AF.Exp, scale=-1.0)
                s1 = scr.tile([128, HB, CI, 8], BF16, name="s1", tag="F")
                nc.vector.tensor_tensor(
                    s1[:], c_t[:, :, :, 0:8], c_t[:, :, :, 8:16], op=ALU.add
                )
                s2 = scr.tile([128, HB, CI, 4], BF16, name="s2", tag="G")
                nc.vector.tensor_tensor(
                    s2[:], s1[:, :, :, 0:4], s1[:, :, :, 4:8], op=ALU.add
                )
                s3 = scr.tile([128, HB, CI, 2], BF16, name="s3", tag="HH")
                nc.vector.tensor_tensor(
                    s3[:], s2[:, :, :, 0:2], s2[:, :, :, 2:4], op=ALU.add
                )
                nc.vector.tensor_tensor(
                    s_tb[:, 0:18], s3[:, :, :, 0], s3[:, :, :, 1], op=ALU.add
                )
                pe_sum_pools()
                nc.vector.reciprocal(rcp_t[:], sumc_t[:])
                nc.vector.tensor_copy(rcpb_t[:], rcp_t[:])
                rb = rcpb_t[:].unsqueeze(3).broadcast_to([128, HB, CI, CO])
                tt_split(0, HB,
                         lambda a, b: c_t[:, a:b],
                         lambda a, b: c_t[:, a:b],
                         lambda a, b: rb[:, a:b], ALU.mult, pool_hb=4)
                r_b = c_t[:].unsqueeze(3).broadcast_to([128, HB, CI, DO, CO])
                for (h0, h1) in CHUNKS:
                    for eng, lo, hi, kd in parts(h0, h1):
                        n = hi - lo
                        x = scr.tile([128, n, CI, DO, CO], BF16,
                                     name="x" + kd, tag="A" + kd)
                        eng.tensor_tensor(x[:], uhat[:, lo:hi], r_b[:, lo:hi],
                                          op=ALU.mult)
                        t1 = scr.tile([128, n, 4, DO, CO], BF16,
                                      name="t1b" + kd, tag="B" + kd)
                        eng.tensor_tensor(t1[:], x[:, :, 0:4], x[:, :, 4:8],
                                          op=ALU.add)
                        t2 = scr.tile([128, n, 2, DO, CO], BF16,
                                      name="t2b" + kd, tag="C" + kd)
                        eng.tensor_tensor(t2[:], t1[:, :, 0:2], t1[:, :, 2:4],
                                          op=ALU.add)
                        eng.tensor_tensor(p_t[:, lo:hi], t2[:, :, 0],
                                          t2[:, :, 1], op=ALU.add)

            # ---- squash: p -> v ----
            nc.scalar.activation(dum[:], eps_t[:], AF.Sqrt)
            for (h0, h1) in (CHUNKS if last else [(0, HB)]):
                n = h1 - h0
                sq = scr.tile([128, n, DO, CO], BF16, name="sq", tag="D")
                nc.scalar.activation(sq[:], p_t[:, h0:h1], AF.Square)
                n1 = scr.tile([128, n, 8, CO], BF16, name="n1", tag="F")
                tt_split(h0, h1,
                         lambda a, b: n1[:, a - h0 : b - h0],
                         lambda a, b: sq[:, a - h0 : b - h0, 0:8],
                         lambda a, b: sq[:, a - h0 : b - h0, 8:16], ALU.add,
                         pool_hb=0 if last else 3)
                n2 = scr.tile([128, n, 4, CO], BF16, name="n2", tag="G")
                nc.vector.tensor_tensor(n2[:], n1[:, :, 0:4], n1[:, :, 4:8],
                                        op=ALU.add)
                n3 = scr.tile([128, n, 2, CO], BF16, name="n3", tag="HH")
                nc.vector.tensor_tensor(n3[:], n2[:, :, 0:2], n2[:, :, 2:4],
                                        op=ALU.add)
                nc.vector.tensor_tensor(nsq_t[:, h0:h1], n3[:, :, 0],
                                        n3[:, :, 1], op=ALU.add)
            nc.scalar.activation(rs_t[:], nsq_t[:], AF.Sqrt, bias=eps_t[:])
            nc.vector.scalar_tensor_tensor(
                rs_t[:], nsq_t[:], 1.0, rs_t[:], op0=ALU.add, op1=ALU.mult
            )
            nc.vector.reciprocal(rs_t[:], rs_t[:])
            nc.vector.tensor_tensor(nsq_t[:], nsq_t[:], rs_t[:], op=ALU.mult)
            nc.vector.tensor_copy(g2b_t[:], nsq_t[:])
            g_b = g2b_t[:].unsqueeze(2).broadcast_to([128, HB, DO, CO])
            if last:
                tt_split(0, HB,
                         lambda a, b: v_bf[:, a:b],
                         lambda a, b: p_t[:, a:b],
                         lambda a, b: g_b[:, a:b], ALU.mult, pool_hb=4)

            if not last:
                nc.scalar.activation(dum[:], eps_t[:], AF.Exp)
                p_b = p_t[:].unsqueeze(2).broadcast_to([128, HB, CI, DO, CO])
                for (h0, h1) in CHUNKS:
                    for eng, lo, hi, kd in parts(h0, h1):
                        n = hi - lo
                        y = scr.tile([128, n, CI, DO, CO], BF16,
                                     name="y" + kd, tag="A" + kd)
                        eng.tensor_tensor(y[:], uhat[:, lo:hi], p_b[:, lo:hi],
                                          op=ALU.mult)
                        e1 = scr.tile([128, n, CI, 8, CO], BF16,
                                      name="e1" + kd, tag="B" + kd)
                        eng.tensor_tensor(e1[:], y[:, :, :, 0:8],
                                          y[:, :, :, 8:16], op=ALU.add)
                        e2 = scr.tile([128, n, CI, 4, CO], BF16,
                                      name="e2" + kd, tag="C" + kd)
                        eng.tensor_tensor(e2[:], e1[:, :, :, 0:4],
                                          e1[:, :, :, 4:8], op=ALU.add)
                        e3 = scr.tile([128, n, CI, 2, CO], BF16,
                                      name="e3" + kd, tag="J" + kd)
                        eng.tensor_tensor(e3[:], e2[:, :, :, 0:2],
                                          e2[:, :, :, 2:4], op=ALU.add)
                        db = scr.tile([128, n, CI, CO], BF16,
                                      name="db" + kd, tag="K" + kd)
                        eng.tensor_tensor(db[:], e3[:, :, :, 0],
                                          e3[:, :, :, 1], op=ALU.add)
                        g2_b = g2b_t[:, lo:hi].unsqueeze(2).broadcast_to(
                            [128, n, CI, CO])
                        if it == 0:
                            eng.tensor_tensor(b_t[:, lo:hi], db[:], g2_b,
                                              op=ALU.mult)
                        else:
                            eng.tensor_tensor(db[:], db[:], g2_b, op=ALU.mult)
                            eng.tensor_tensor(b_t[:, lo:hi], b_t[:, lo:hi],
                                              db[:], op=ALU.add)
                    n = h1 - h0
                    u1 = scr.tile([128, n, CI, 8], F32, name="u1", tag="Eu")
                    nc.vector.tensor_tensor(
                        u1[:], b_t[:, h0:h1, :, 0:8], b_t[:, h0:h1, :, 8:16],
                        op=ALU.max,
                    )
                    u2 = scr.tile([128, n, CI, 4], F32, name="u2", tag="Fu")
                    nc.vector.tensor_tensor(
                        u2[:], u1[:, :, :, 0:4], u1[:, :, :, 4:8], op=ALU.max
                    )
                    u3 = scr.tile([128, n, CI, 2], F32, name="u3", tag="Gu")
                    nc.vector.tensor_tensor(
                        u3[:], u2[:, :, :, 0:2], u2[:, :, :, 2:4], op=ALU.max
                    )
                    nc.vector.tensor_tensor(
                        m0C_t[:, h0:h1], u3[:, :, :, 0], u3[:, :, :, 1],
                        op=ALU.max,
                    )
                    nc.vector.tensor_scalar(
                        m0C_t[:, h0:h1], m0C_t[:, h0:h1], CC, None,
                        op0=ALU.add,
                    )
            else:
                for (h0, h1) in CHUNKS:
                    nc.sync.dma_start(v_d[:, h0:h1], v_bf[:, h0:h1])
    return nc


# ============================ host side ============================

_CACHE = {}


def _host_consts(w):
    # w: [Ci, Co*Do, Di, 5, 5] f32, channel index = co*16+do.
    # Conv lhsT rows: pat1 row = di*16 + kh*4 + kw (kh,kw in 0..4);
    # pat2 row = di*5 + kw for (kh=4, kw 0..5), then 40 + di*4 + kh for
    # (kh 0..4, kw=4).  Columns m = do*16 + co.
    w4 = w.reshape(CI, CO, DO, DI, 5, 5).transpose(3, 4, 5, 0, 2, 1)
    # w4: [di, kh, kw, ci, do, co]
    w4 = np.ascontiguousarray(w4).reshape(DI, 5, 5, CI, 256)
    w1 = np.ascontiguousarray(
        w4[:, 0:4, 0:4].reshape(128, CI, 256)
    ).astype(ml_dtypes.bfloat16)
    w2a = w4[:, 4, 0:5].reshape(40, CI, 256)
    w2b = w4[:, 0:4, 4].reshape(32, CI, 256)
    w2 = np.ascontiguousarray(np.concatenate([w2a, w2b], 0)).astype(
        ml_dtypes.bfloat16
    )

    hw_cnt = np.zeros((H, W), np.float32)
    for h in range(H):
        for wv in range(W):
            ch = min(h + 2, H - 1) - max(h - 2, 0) + 1
            cw = min(wv + 2, W - 1) - max(wv - 2, 0) + 1
            hw_cnt[h, wv] = ch * cw
    r0 = 1.0 / (CO * hw_cnt)
    r0c = np.ascontiguousarray(r0.reshape(HB, 128).T)
    return w1, w2, r0c


def _im2col(un):
    """un: [Ci, Di, H, W] bf16 -> pat1 [Ci, 128, HW], pat2 [Ci, 72, HW] bf16.
    Row layouts match _host_consts."""
    up = np.zeros((CI, DI, H + 4, W + 4), ml_dtypes.bfloat16)
    up[:, :, 2 : 2 + H, 2 : 2 + W] = un
    sw = np.lib.stride_tricks.sliding_window_view(up, (H, W), axis=(2, 3))
    # sw: [Ci, Di, 5, 5, H, W]
    p1 = sw[:, :, 0:4, 0:4].reshape(CI, DI * 16, HW)
    p2a = sw[:, :, 4, 0:5].reshape(CI, DI * 5, HW)
    p2b = sw[:, :, 0:4, 4].reshape(CI, DI * 4, HW)
    p2 = np.concatenate([p2a, p2b], 1)
    return np.ascontiguousarray(p1), np.ascontiguousarray(p2)


def _get_nc():
    if "nc" not in _CACHE:
        nc = bacc.Bacc("TRN2", target_bir_lowering=False, debug=False, num_devices=8)
        _emit(nc)
        nc.compile()
        _CACHE["nc"] = nc
    return _CACHE["nc"]


def kernel(u, w):
    u = np.asarray(u, np.float32)
    N = u.shape[0]
    assert N == 8
    nc = _get_nc()
    w1, w2, r0c = _host_consts(np.asarray(w, np.float32))
    ub = u.astype(ml_dtypes.bfloat16)
    in_maps = []
    for n in range(N):
        p1, p2 = _im2col(ub[n])
        in_maps.append({"p1": p1, "p2": p2, "w1": w1, "w2": w2, "r0c": r0c})
    res = run_bass_kernel_spmd(nc, in_maps, core_ids=list(range(N)))
    out = np.stack(
        [res.results[n]["v"].astype(np.float32) for n in range(N)]
    )  # [8, 128, HB, DO, CO]
    # hw = hb*128 + p ; out[n, co, do, h, w]
    out = out.transpose(0, 2, 1, 3, 4).reshape(N, HW, DO, CO)
    out = out.reshape(N, H, W, DO, CO).transpose(0, 4, 3, 1, 2)
    return np.ascontiguousarray(out, dtype=np.float32)
